# revision 23
# baseline (speedup 1.0000x reference)
"""Adaptive-softmax cross-entropy loss on 8 Trainium2 NeuronCores.

Strategy (token-parallel with label-sorted routing):
  * Tokens are sorted by label on the host and dealt round-robin so each
    core takes 512 head tokens plus its share of the cluster-0/1 token
    runs for the two tail softmaxes (low-rank input projections).
  * Each core gathers its token rows from HBM with a transposing
    dma_gather (bf16), landing directly in [K,128]-tile layout for the
    TensorEngine; gpsimd converts to fp8 for DoubleRow matmuls.
  * Streaming softmax over 2048-wide vocab chunks; logits need no
    max-subtraction (|logit| ~ 4).  Per-chunk sum-of-exp is computed on
    TWO engines in parallel, chosen per chunk by a static pattern:
      - ScalarE: Exp activation with fused accum_out.
      - VectorE: Schraudolph fast-exp in bf16-bit space: int16(A*l + B)
        bit-reinterpreted as bf16 ~= exp(l), then a 4x-mode accumulate
        pass.  B tuned so the sum-ratio bias is ~zero.
  * The label logit is NOT extracted from the logit stream.  Instead a
    per-token dot product x . W[:, label] is computed directly: gather
    W^T rows by label (dma_gather) and multiply-accumulate against a
    token-major x gather on VectorE.  For tails the projected hidden
    (256/64-dim, already token-major on chip) is used.
  * Device outputs per-token (sum_exp, label_logit); host finishes with
    log() and the masked mean (tiny O(tokens) work).
"""

import math
import os as _os
from contextlib import ExitStack, nullcontext

import numpy as np
import ml_dtypes

import concourse.bass as bass
import concourse.mybir as mybir
import concourse.tile as tile
from concourse import bacc
from concourse.bass_utils import run_bass_kernel_spmd
from concourse.masks import make_identity

CUTOFFS = (16000, 28000, 36000)
HID = 1024
NCORES = 8
CH = int(_os.environ.get("KERNEL_CH", "1024"))  # vocab chunk width
PSUM_BUFS = int(_os.environ.get("KERNEL_PSUM_BUFS", "4"))
BF16 = mybir.dt.bfloat16
FP8 = mybir.dt.float8e4
F32 = mybir.dt.float32
I16 = mybir.dt.int16
NPBF16 = ml_dtypes.bfloat16
NPFP8 = ml_dtypes.float8_e4m3

# Schraudolph fast-exp in bf16 bit space: bf16_bits(int16(A16*l + B16)) ~ exp(l).
# B16 sits between the round- and trunc-tuned optima so either hardware
# conversion mode keeps the sum-ratio bias under ~1.5e-3.
A16 = float(128.0 / math.log(2.0))
B16 = float(_os.environ.get("KERNEL_B16", "16248.635"))

# which chunk instances the VectorE Schraudolph path takes (i mod 12)
DVE_SLOTS = tuple(
    int(x) for x in _os.environ.get("KERNEL_DVE_SLOTS", "1,3,6,8,10").split(",") if x != ""
)
T1_BF16 = _os.environ.get("KERNEL_T1_BF16", "0") == "1"  # bisect switch
NO_LLDOT = _os.environ.get("KERNEL_NO_LLDOT", "0") == "1"  # bisect switch
CONV_ENGINE = _os.environ.get("KERNEL_CONV", "pool")  # fp8 convert engine
WEAVE = _os.environ.get("KERNEL_WEAVE", "1") == "1"  # merged vs sequential

# ---------------------------------------------------------------------------
# Workaround for this container's walrus build: CoreV3 codegen accepts only
# ONE embedded sync-wait per instruction, while Tile emits instructions whose
# sync_info carries one wait per producing logical processor. Legalize after
# scheduling: hoist all-but-one wait onto same-engine NoOps inserted directly
# before the instruction (same-engine program order makes this equivalent).
_nop_counter = [0]


def _legalize_sync_waits(nc, max_waits=1):
    for fn in nc.m.functions:
        for blk in fn.blocks:
            insts = blk.instructions
            if not any(
                inst.sync_info is not None
                and inst.sync_info.on_wait
                and len(inst.sync_info.on_wait) > max_waits
                for inst in insts
            ):
                continue
            new = []
            for inst in insts:
                si = inst.sync_info
                waits = list(si.on_wait) if (si is not None and si.on_wait) else []
                if len(waits) > max_waits:
                    for w in waits[:-max_waits]:
                        _nop_counter[0] += 1
                        nop = mybir.InstNoOp(
                            name=f"LW-{_nop_counter[0]}", ins=[], outs=[]
                        )
                        nop.engine = inst.engine
                        nop.sync_info = mybir.SyncInfo(on_wait=[w], on_update=[])
                        nc.register_instruction(nop, overwrite=True)
                        new.append(nop)
                    inst.sync_info = mybir.SyncInfo(
                        on_wait=waits[-max_waits:],
                        on_update=list(si.on_update) if si.on_update else [],
                    )
                new.append(inst)
            blk.instructions = new
# ---------------------------------------------------------------------------


def _cdiv(a, b):
    return (a + b - 1) // b


def _make_chunks(V, prime=False):
    """Vocab chunk boundaries. prime=True starts with narrow chunks so the
    first matmul/exp of the stream fires before the full 2048-wide weight
    chunk has landed (pipeline priming)."""
    chunks = []
    v0 = 0
    if prime and V > 2048:
        chunks = [(0, 1024), (1024, 1024)]
        v0 = 2048
    while v0 < V:
        vw = min(CH, V - v0)
        chunks.append((v0, vw))
        v0 += vw
    return tuple(chunks)


def _wrap_idxs(idxs, num):
    """dma_gather index layout: idx i lives at [i % 16, i // 16], and the
    16-partition block is replicated to all 8 gpsimd cores (128 partitions)."""
    assert num % 16 == 0 and len(idxs) == num
    a = np.asarray(idxs, np.int16).reshape(num // 16, 16).T  # [16, num/16]
    return np.tile(a, (8, 1))  # [128, num/16]


def build_graph(plan, reps=1):
    """One SPMD graph, identical for all 8 cores.

    reps > 1 unrolls the whole kernel body back-to-back inside the NEFF so
    a timing harness can measure marginal (steady-state) per-rep cost,
    cancelling host/dispatch overhead: t = (T(K) - T(1)) / (K - 1)."""
    ntok = plan["ntok"]
    tpc = plan["tpc"]  # head tokens per core (multiple of 128)
    cap0, cap1 = plan["cap0"], plan["cap1"]  # tail token capacity per core
    nbh, nb0, nb1 = tpc // 128, cap0 // 128, cap1 // 128
    chh, ch0l, ch1l = plan["chunks_h"], plan["chunks_0"], plan["chunks_1"]
    nchh, nch0, nch1 = len(chh), len(ch0l), len(ch1l)
    use_bias = plan["use_bias"]
    ncols = plan["ncols"]
    nse = nbh + nb0 + nb1  # sum_exp columns; label-logit columns follow

    nc = bacc.Bacc(num_devices=NCORES)

    xt = nc.declare_dram_parameter("xt", [ntok, HID], BF16, isOutput=False)
    hw = nc.declare_dram_parameter("hw", [HID, 16002], FP8, isOutput=False)
    p0 = nc.declare_dram_parameter("p0", [HID, 256], FP8, isOutput=False)
    w0 = nc.declare_dram_parameter("w0", [256, 12000], FP8, isOutput=False)
    p1 = nc.declare_dram_parameter("p1", [HID, 64], BF16, isOutput=False)
    w1 = nc.declare_dram_parameter("w1", [64, 8000], BF16 if T1_BF16 else FP8, isOutput=False)
    hwT = nc.declare_dram_parameter("hwT", [16002, HID], BF16, isOutput=False)
    w0T = nc.declare_dram_parameter("w0T", [12000, 256], BF16, isOutput=False)
    w1T = nc.declare_dram_parameter("w1T", [8000, 128], BF16, isOutput=False)
    hidx = nc.declare_dram_parameter("hidx", [128, tpc // 16], I16, isOutput=False)
    idx0 = nc.declare_dram_parameter("idx0", [128, cap0 // 16], I16, isOutput=False)
    idx1 = nc.declare_dram_parameter("idx1", [128, cap1 // 16], I16, isOutput=False)
    lidxh = nc.declare_dram_parameter("lidxh", [128, tpc // 16], I16, isOutput=False)
    lidx0 = nc.declare_dram_parameter("lidx0", [128, cap0 // 16], I16, isOutput=False)
    lidx1 = nc.declare_dram_parameter("lidx1", [128, cap1 // 16], I16, isOutput=False)
    if use_bias:
        hb = nc.declare_dram_parameter("hb", [1, 16002], BF16, isOutput=False)
        b0 = nc.declare_dram_parameter("b0", [1, 12000], BF16, isOutput=False)
        b1 = nc.declare_dram_parameter("b1", [1, 8000], BF16, isOutput=False)
        pb0 = nc.declare_dram_parameter("pb0", [1, 256], BF16, isOutput=False)
        pb1 = nc.declare_dram_parameter("pb1", [1, 64], BF16, isOutput=False)
    out = nc.declare_dram_parameter("out", [128, ncols], F32, isOutput=True)
    if _os.environ.get("KERNEL_DEBUG_SE", "0") == "1":
        dbg_hse = nc.declare_dram_parameter("dbg_hse", [128, nchh * 8], F32, isOutput=True)

    Exp = mybir.ActivationFunctionType.Exp
    Mult = mybir.AluOpType.mult
    Add = mybir.AluOpType.add

    with tile.TileContext(nc) as tc:
        with ExitStack() as ctx:
            const = ctx.enter_context(tc.tile_pool(name="const", bufs=1))
            wpool = ctx.enter_context(tc.tile_pool(name="w", bufs=4))
            spool = ctx.enter_context(tc.tile_pool(name="scratch", bufs=1))

            # --- setup: indices / constants (outside rep loop) ---
            idx_sb = {}
            for name, ap, n in (
                ("hidx", hidx, tpc), ("idx0", idx0, cap0), ("idx1", idx1, cap1),
                ("lidxh", lidxh, tpc), ("lidx0", lidx0, cap0), ("lidx1", lidx1, cap1),
            ):
                t = const.tile([128, n // 16], I16, name=f"sb_{name}")
                nc.sync.dma_start(out=t[:, :], in_=ap[:, :])
                idx_sb[name] = t

            identity = const.tile([128, 128], BF16)
            make_identity(nc, identity[:, :])
            ones1 = const.tile([1, 128], BF16)
            nc.vector.memset(ones1[:, :], 1.0)

            def emit_proj_consts():
                nc.sync.dma_start(out=p0_sb[:, :, :],
                                  in_=p0.ap().rearrange("(c p) n -> p c n", p=128))
                nc.sync.dma_start(out=p1_sb[:, :, :],
                                  in_=p1.ap().rearrange("(c p) n -> p c n", p=128))

            bias_sb = {}
            if use_bias:
                for name, ap, n in (
                    ("hb", hb, 16002), ("b0", b0, 12000), ("b1", b1, 8000),
                    ("pb0", pb0, 256), ("pb1", pb1, 64),
                ):
                    t = const.tile([1, n], BF16, tag=f"bias_{name}")
                    nc.sync.dma_start(out=t[:, :], in_=ap[:, :])
                    bias_sb[name] = t

            p0_sb = const.tile([128, 8, 256], FP8)
            p1_sb = const.tile([128, 8, 64], BF16)

            # persistent result tiles (rewritten each rep); xh is block-major
            # [128, nbh, 8K, 128tok] so per-block gathers stay contiguous
            xh = const.tile([128, nbh, 8, 128], BF16)
            x0 = const.tile([128, 8, cap0], BF16)
            x1 = const.tile([128, 8, cap1], BF16)
            xh8 = const.tile([128, nbh, 8, 128], FP8)
            x08 = const.tile([128, 8, cap0], FP8)
            xtm = const.tile([128, nbh, HID], BF16)   # token-major x rows
            wgh = const.tile([128, nbh, HID], BF16)   # W^T rows by head label
            wg0 = const.tile([128, nb0, 256], BF16)
            wg1 = const.tile([128, nb1, 128], BF16)
            hsb0 = const.tile([128, nb0, 256], BF16)  # token-major projected h
            hsb1 = const.tile([128, nb1, 64], BF16)
            hT0 = const.tile([128, 2, cap0], FP8)
            hT1 = const.tile([64, 1, cap1], BF16 if T1_BF16 else FP8)
            hse = const.tile([128, nbh, nchh], F32)
            se0 = const.tile([128, nb0, nch0], F32)
            se1 = const.tile([128, nb1, nch1], F32)
            out_sb = const.tile([128, 2, ncols], F32)

            chunk_counter = [0]
            last_eng = ["D"]
            fillers = []
            conv_eng = nc.gpsimd if CONV_ENGINE == "pool" else nc.vector

            def emit_body(rep):
                par = rep % 2
                # gather this core's token rows (transposed, bf16) and the
                # label-indexed W^T rows. Head gathers go per 128-token block
                # so the first matmuls start early; gpsimd converts to fp8
                # right after each gather.
                with tc.high_priority() if rep == 0 else nullcontext():
                    for b in range(nbh):
                        nc.gpsimd.dma_gather(
                            xh[:, b, :, :], xt[:, :],
                            idx_sb["hidx"][:, b * 8 : (b + 1) * 8],
                            num_idxs=128, num_idxs_reg=128, elem_size=HID,
                            transpose=True,
                        )
                        conv_eng.tensor_copy(xh8[:, b, :, :], xh[:, b, :, :])
                nc.gpsimd.dma_gather(
                    x0[:, :, :], xt[:, :], idx_sb["idx0"][:, :],
                    num_idxs=cap0, num_idxs_reg=cap0, elem_size=HID, transpose=True,
                )
                conv_eng.tensor_copy(x08[:, :, :], x0[:, :, :])
                nc.gpsimd.dma_gather(
                    x1[:, :, :], xt[:, :], idx_sb["idx1"][:, :],
                    num_idxs=cap1, num_idxs_reg=cap1, elem_size=HID, transpose=True,
                )
                nc.gpsimd.dma_gather(
                    xtm[:, :, :], xt[:, :], idx_sb["hidx"][:, :],
                    num_idxs=tpc, num_idxs_reg=tpc, elem_size=HID, transpose=False,
                )
                nc.gpsimd.dma_gather(
                    wgh[:, :, :], hwT[:, :], idx_sb["lidxh"][:, :],
                    num_idxs=tpc, num_idxs_reg=tpc, elem_size=HID, transpose=False,
                )
                nc.gpsimd.dma_gather(
                    wg0[:, :, :], w0T[:, :], idx_sb["lidx0"][:, :],
                    num_idxs=cap0, num_idxs_reg=cap0, elem_size=256, transpose=False,
                )
                nc.gpsimd.dma_gather(
                    wg1[:, :, :], w1T[:, :], idx_sb["lidx1"][:, :],
                    num_idxs=cap1, num_idxs_reg=cap1, elem_size=128, transpose=False,
                )

                # --- tail projections: h = x @ pW (+pb) -> transpose [proj, tok]
                # Emitted AFTER the head stream (see below): engines execute
                # their instruction streams in order, so anything placed
                # before the head matmuls would stall PE on the tail gathers.
                # Projection PSUM tiles borrow the "logits" slots.
                def emit_proj(ppsum):
                    for tb in range(nb0):
                        ph_t = ppsum.tile([128, CH], F32, tag="logits", name="ph_t")
                        ph = ph_t[:, :256]
                        for c2 in range(4):
                            nc.tensor.matmul(
                                ph[:, :],
                                x08[:, 2 * c2 : 2 * c2 + 2, bass.ts(tb, 128)],
                                p0_sb[:, 2 * c2 : 2 * c2 + 2, :],
                                start=(c2 == 0), stop=(c2 == 3 and not use_bias),
                                perf_mode=mybir.MatmulPerfMode.DoubleRow,
                            )
                        if use_bias:
                            nc.tensor.matmul(
                                ph[:, :], ones1[0:1, :], bias_sb["pb0"][0:1, :],
                                start=False, stop=True,
                            )
                        nc.scalar.copy(hsb0[:, tb, :], ph[:, :])
                        for j in range(2):
                            pt_t = ppsum.tile([128, CH], BF16, tag="logits", name="pt_t")
                            pt = pt_t[:, :128]
                            nc.tensor.transpose(pt[:, :], hsb0[:, tb, bass.ts(j, 128)], identity[:, :])
                            nc.scalar.copy(hT0[:, j, bass.ts(tb, 128)], pt[:, :])
                    for tb in range(nb1):
                        ph_t = ppsum.tile([128, CH], F32, tag="logits", name="ph_t")
                        ph = ph_t[:, :64]
                        for c in range(8):
                            nc.tensor.matmul(
                                ph[:, :], x1[:, c, bass.ts(tb, 128)], p1_sb[:, c, :],
                                start=(c == 0), stop=(c == 7 and not use_bias),
                            )
                        if use_bias:
                            nc.tensor.matmul(
                                ph[:, :], ones1[0:1, :], bias_sb["pb1"][0:1, :],
                                start=False, stop=True,
                            )
                        nc.scalar.copy(hsb1[:, tb, :], ph[:, :])
                        pt_t = ppsum.tile([128, CH], BF16, tag="logits", name="pt_t")
                        pt = pt_t[:64, :128]
                        nc.tensor.transpose(pt[:, :], hsb1[:, tb, :], identity[:, :])
                        nc.scalar.copy(hT1[:, 0, bass.ts(tb, 128)], pt[:, :])

                # label-logit dot products: ll[tok] = sum(a[tok,:] * wg[tok,:])
                def lldot_op(a_sb, wg_sb, j, width, col0):
                    def emit():
                        prod = spool.tile([128, HID], BF16, tag="prod", name="prod")
                        nc.vector.tensor_tensor(
                            out=prod[:, :width], in0=a_sb[:, j, :width],
                            in1=wg_sb[:, j, :width], op=Mult)
                        junk2 = spool.tile([128, HID], BF16, tag="junk2", name="junk2")
                        nc.vector.tensor_scalar(
                            out=junk2[:, :width], in0=prod[:, :width],
                            scalar1=1.0, scalar2=None, op0=Mult, op1=Add,
                            accum_out=out_sb[:, par, nse + col0 + j : nse + col0 + j + 1],
                        )
                    return emit

                # --- streaming softmax over vocab chunks, three logical
                # streams (head / tail0 / tail1) interleaved into one merged
                # schedule so PE-heavy head chunks overlap the consumer-heavy
                # tail chunks and the weight DMA never idles.
                def emit_chunk(st, ch, hook=None):
                    KT, kpart, Wr, chunks, nb, se_sb, bias, dr, xsl = (
                        st["KT"], st["kpart"], st["Wr"], st["chunks"],
                        st["nb"], st["se"], st["bias"], st["dr"], st["xsl"])
                    v0, vw = chunks[ch]
                    wdt = FP8 if st["fp8"] else BF16
                    wt = wpool.tile([kpart, KT, vw], wdt, tag="wt", name="wt")
                    if ch == 0 and KT > 1:
                        # split per K-pair so the first matmul needs only
                        # 1/(KT/2) of the chunk's weights (subtile deps)
                        kstep = 2 if dr else 1
                        for c2 in range(KT // kstep):
                            nc.sync.dma_start(
                                out=wt[:, kstep * c2 : kstep * (c2 + 1), :],
                                in_=Wr[:, kstep * c2 : kstep * (c2 + 1), v0 : v0 + vw],
                            )
                    else:
                        nc.sync.dma_start(out=wt[:, :, :], in_=Wr[:, :, v0 : v0 + vw])
                    if hook is not None:
                        hook()
                    for tb in range(nb):
                        ps = lpsum.tile([128, CH], F32, tag="logits", name="ps")
                        if dr:
                            for c2 in range(KT // 2):
                                for s0 in range(0, vw, 512):
                                    sw = min(512, vw - s0)
                                    nc.tensor.matmul(
                                        ps[:, s0 : s0 + sw],
                                        xsl(tb, 2 * c2, 2 * c2 + 2),
                                        wt[:, 2 * c2 : 2 * c2 + 2, s0 : s0 + sw],
                                        start=(c2 == 0),
                                        stop=(c2 == KT // 2 - 1 and bias is None),
                                        perf_mode=mybir.MatmulPerfMode.DoubleRow,
                                    )
                        else:
                            for c in range(KT):
                                for s0 in range(0, vw, 512):
                                    sw = min(512, vw - s0)
                                    nc.tensor.matmul(
                                        ps[:, s0 : s0 + sw],
                                        xsl(tb, c, c + 1),
                                        wt[:, c, s0 : s0 + sw],
                                        start=(c == 0),
                                        stop=(c == KT - 1 and bias is None),
                                    )
                        if bias is not None:
                            for s0 in range(0, vw, 512):
                                sw = min(512, vw - s0)
                                nc.tensor.matmul(
                                    ps[:, s0 : s0 + sw],
                                    ones1[0:1, :],
                                    bias[0:1, v0 + s0 : v0 + s0 + sw],
                                    start=False, stop=True,
                                )
                        # sum-of-exp: ScalarE Exp or VectorE Schraudolph,
                        # chosen per chunk instance by a static pattern.
                        # On an ACT-ACT double (VectorE bubble), slip one
                        # pending DVE filler op (ll-dot / partial reduce)
                        # into the VectorE stream.
                        idx = chunk_counter[0]
                        chunk_counter[0] += 1
                        if (idx % 12) in DVE_SLOTS:
                            last_eng[0] = "D"
                            i16t = spool.tile([128, CH], I16, tag="i16", name="i16t")
                            nc.vector.tensor_scalar(
                                out=i16t[:, :vw], in0=ps[:, :vw],
                                scalar1=A16, scalar2=B16, op0=Mult, op1=Add,
                            )
                            junk = spool.tile([128, CH], BF16, tag="junk", name="junk")
                            nc.vector.tensor_scalar(
                                out=junk[:, :vw], in0=i16t.bitcast(BF16)[:, :vw],
                                scalar1=1.0, scalar2=None, op0=Mult, op1=Add,
                                accum_out=se_sb[:, tb, ch : ch + 1],
                            )
                        else:
                            if last_eng[0] == "A" and fillers:
                                fillers.pop(0)()
                            last_eng[0] = "A"
                            ex = spool.tile([128, CH], BF16, tag="ex", name="ex")
                            nc.scalar.activation(
                                ex[:, :vw], ps[:, :vw], Exp,
                                accum_out=se_sb[:, tb, ch : ch + 1],
                            )

                def reduce_op(se_sb, tb, col):
                    def emit():
                        nc.vector.reduce_sum(
                            out=out_sb[:, par, col + tb : col + tb + 1],
                            in_=se_sb[:, tb, :], axis=mybir.AxisListType.X,
                        )
                    return emit

                def blocked_xsl(tb, ca, cb):
                    return xh8[:, tb, ca:cb, :]

                def flat_xsl(xT):
                    return lambda tb, ca, cb: xT[:, ca:cb, bass.ts(tb, 128)]

                sth = dict(KT=8, kpart=128, Wr=hw.ap().rearrange("(c p) n -> p c n", p=128),
                           chunks=chh, nb=nbh, se=hse, bias=bias_sb.get("hb"),
                           dr=True, fp8=True, xsl=blocked_xsl)
                st0 = dict(KT=2, kpart=128, Wr=w0.ap().rearrange("(c p) n -> p c n", p=128),
                           chunks=ch0l, nb=nb0, se=se0, bias=bias_sb.get("b0"),
                           dr=True, fp8=True, xsl=flat_xsl(hT0))
                st1 = dict(KT=1, kpart=64, Wr=w1.ap().rearrange("(c p) n -> p c n", p=64),
                           chunks=ch1l, nb=nb1, se=se1, bias=bias_sb.get("b1"),
                           dr=False, fp8=not T1_BF16, xsl=flat_xsl(hT1))

                with tc.tile_pool(name="lpsum", bufs=PSUM_BUFS, space="PSUM") as lpsum:
                    # priming: two narrow head chunks to get PE going fast
                    emit_chunk(sth, 0, hook=emit_proj_consts if rep == 0 else None)
                    emit_chunk(sth, 1)
                    # projections early: hT0/hT1/hsb ready before their chunks
                    emit_proj(lpsum)
                    # merged schedule: weave tail chunks between head chunks
                    heads = [("h", c) for c in range(2, nchh)]
                    # interleave t1 chunks evenly among t0 chunks
                    tails = []
                    i0 = i1 = 0
                    for i in range(nch0 + nch1):
                        if i1 < ((i + 1) * nch1) // (nch0 + nch1):
                            tails.append(("t1", i1))
                            i1 += 1
                        else:
                            tails.append(("t0", i0))
                            i0 += 1
                    assert i0 == nch0 and i1 == nch1
                    if WEAVE:
                        # weave tails between heads, evenly spread
                        weave = []
                        nt = len(tails)
                        nh = len(heads)
                        ti = 0
                        for i, h in enumerate(heads):
                            weave.append(h)
                            want = (i + 1) * nt // nh
                            while ti < want:
                                weave.append(tails[ti])
                                ti += 1
                        weave.extend(tails[ti:])
                    else:
                        weave = heads + tails

                    if not NO_LLDOT:
                        head_fill = [lldot_op(xtm, wgh, j, HID, 0) for j in range(nbh)]
                        tail_fill = (
                            [lldot_op(hsb0, wg0, j, 256, nbh) for j in range(nb0)]
                            + [lldot_op(hsb1, wg1, j, 64, nbh + nb0) for j in range(nb1)]
                        )
                    else:
                        head_fill, tail_fill = [], []
                    for i, (s, ch) in enumerate(weave):
                        emit_chunk({"h": sth, "t0": st0, "t1": st1}[s], ch)
                        if i == 0 and pending_out:
                            pending_out.pop(0)()
                        if i == 1:
                            # gathers have landed; ll-dots become available
                            fillers.extend(head_fill)
                        elif i == 3:
                            # projections done; tail ll-dots become available
                            fillers.extend(tail_fill)
                        elif s == "t0" and ch == nch0 - 1:
                            fillers.extend(reduce_op(se0, tb, nbh) for tb in range(nb0))
                        elif s == "t1" and ch == nch1 - 1:
                            fillers.extend(reduce_op(se1, tb, nbh + nb0) for tb in range(nb1))
                    for tb in range(nbh):
                        reduce_op(hse, tb, 0)()
                    for f in fillers:
                        f()
                    fillers.clear()

                    pending_out.append(
                        lambda p=par: nc.sync.dma_start(out=out[:, :], in_=out_sb[:, p, :]))
                    if _os.environ.get("KERNEL_DEBUG_SE", "0") == "1":
                        nc.sync.dma_start(
                            out=dbg_hse.ap().rearrange("p (b c) -> p b c", b=nbh),
                            in_=hse[:, :, :])

            pending_out = []
            for _rep in range(reps):
                emit_body(_rep)
            pending_out.pop(0)()

    nc.compile()
    _legalize_sync_waits(nc)
    return nc


def make_plan_and_maps(inp, labels, head_W, head_b, t0_pW, t0_pb, t0_W, t0_b,
                       t1_pW, t1_pb, t1_W, t1_b):
    X = np.ascontiguousarray(np.asarray(inp, np.float32).reshape(-1, HID))
    labels = np.asarray(labels).astype(np.int64).reshape(-1)
    ntok = X.shape[0]
    assert ntok % (NCORES * 128) == 0, ntok

    order = np.argsort(labels, kind="stable")
    slab = labels[order]
    head_labels = labels.copy()
    m0 = (labels >= CUTOFFS[0]) & (labels < CUTOFFS[1])
    m1 = (labels >= CUTOFFS[1]) & (labels < CUTOFFS[2])
    head_labels[m0] = CUTOFFS[0]
    head_labels[m1] = CUTOFFS[0] + 1

    tpc = ntok // NCORES
    # Round-robin deal of sorted tokens keeps per-core work symmetric.
    head_tok = [order[c::NCORES] for c in range(NCORES)]

    def split_cluster(lo, hi):
        toks = order[(slab >= lo) & (slab < hi)]  # sorted by label
        n = len(toks)
        per = _cdiv(max(n, 1), NCORES)
        cap = max(_cdiv(per, 128) * 128, 128)
        chunks, valid = [], []
        for c in range(NCORES):
            chunk = toks[c::NCORES]
            v = len(chunk)
            pad = np.zeros(cap - v, np.int64)
            chunks.append(np.concatenate([chunk, pad]))
            valid.append(v)
        return chunks, valid, cap

    c0_tok, c0_valid, cap0 = split_cluster(CUTOFFS[0], CUTOFFS[1])
    c1_tok, c1_valid, cap1 = split_cluster(CUTOFFS[1], CUTOFFS[2])

    nbh, nb0, nb1 = tpc // 128, cap0 // 128, cap1 // 128
    ncols = 2 * (nbh + nb0 + nb1)

    use_bias = any(
        float(np.abs(np.asarray(b, np.float32)).max()) > 0
        for b in (head_b, t0_b, t1_b, t0_pb, t1_pb)
    )

    chunks_h = _make_chunks(16002, prime=True)
    chunks_0 = _make_chunks(12000)
    chunks_1 = _make_chunks(8000)

    plan = dict(ntok=ntok, tpc=tpc, cap0=cap0, cap1=cap1, ncols=ncols,
                use_bias=use_bias, head_tok=head_tok, c0_tok=c0_tok,
                c1_tok=c1_tok, c0_valid=c0_valid, c1_valid=c1_valid,
                labels=labels, head_labels=head_labels,
                head_b=np.asarray(head_b, np.float64),
                t0_b=np.asarray(t0_b, np.float64),
                t1_b=np.asarray(t1_b, np.float64),
                chunks_h=chunks_h, chunks_0=chunks_0, chunks_1=chunks_1)

    Xb = X.astype(NPBF16)
    W0 = np.asarray(t0_W, np.float32)
    W1 = np.asarray(t1_W, np.float32)
    HW = np.asarray(head_W, np.float32)
    w1T = np.zeros((8000, 128), np.float32)
    w1T[:, :64] = W1.T
    shared = {
        "xt": Xb,
        "hw": HW.astype(NPFP8),
        "p0": np.asarray(t0_pW, np.float32).astype(NPFP8),
        "w0": W0.astype(NPFP8),
        "p1": np.asarray(t1_pW, np.float32).astype(NPBF16),
        "w1": W1.astype(NPBF16 if T1_BF16 else NPFP8),
        "hwT": np.ascontiguousarray(HW.T).astype(NPBF16),
        "w0T": np.ascontiguousarray(W0.T).astype(NPBF16),
        "w1T": w1T.astype(NPBF16),
    }
    if use_bias:
        shared["hb"] = np.asarray(head_b, np.float32).astype(NPBF16)[None, :]
        shared["b0"] = np.asarray(t0_b, np.float32).astype(NPBF16)[None, :]
        shared["b1"] = np.asarray(t1_b, np.float32).astype(NPBF16)[None, :]
        shared["pb0"] = np.asarray(t0_pb, np.float32).astype(NPBF16)[None, :]
        shared["pb1"] = np.asarray(t1_pb, np.float32).astype(NPBF16)[None, :]

    in_maps = []
    for c in range(NCORES):
        m = dict(shared)
        m["hidx"] = _wrap_idxs(head_tok[c], tpc)
        m["idx0"] = _wrap_idxs(c0_tok[c], cap0)
        m["idx1"] = _wrap_idxs(c1_tok[c], cap1)
        m["lidxh"] = _wrap_idxs(head_labels[head_tok[c]], tpc)

        def _lab_idx(tok_list, valid, lo, cap):
            v = np.zeros(cap, np.int64)
            v[:valid] = labels[tok_list[:valid]] - lo
            return _wrap_idxs(v, cap)

        m["lidx0"] = _lab_idx(c0_tok[c], c0_valid[c], CUTOFFS[0], cap0)
        m["lidx1"] = _lab_idx(c1_tok[c], c1_valid[c], CUTOFFS[1], cap1)
        in_maps.append(m)
    return plan, in_maps


def assemble_loss(plan, outs):
    """outs: list of per-core [128, ncols] f32 arrays -> mean loss (f64)."""
    ntok = plan["ntok"]
    labels = plan["labels"]
    tpc = plan["tpc"]
    nbh = tpc // 128
    nb0 = plan["cap0"] // 128
    nb1 = plan["cap1"] // 128
    nse = nbh + nb0 + nb1
    total = 0.0
    for c in range(NCORES):
        o = np.asarray(outs[c], np.float64)
        col = 0
        for tok_list, nb, valid, bias, blab in (
            (plan["head_tok"][c], nbh, tpc, plan["head_b"], plan["head_labels"]),
            (plan["c0_tok"][c], nb0, plan["c0_valid"][c], plan["t0_b"],
             labels - CUTOFFS[0]),
            (plan["c1_tok"][c], nb1, plan["c1_valid"][c], plan["t1_b"],
             labels - CUTOFFS[1]),
        ):
            # block b partition p <-> token b*128+p of tok_list
            se = o[:, col : col + nb].T.reshape(-1)[:valid]
            ll = o[:, nse + col : nse + col + nb].T.reshape(-1)[:valid]
            toks = tok_list[:valid]
            w = (labels[toks] != 0).astype(np.float64)
            lb = bias[blab[toks]] if bias.ndim else 0.0
            total += float(np.dot(w, np.log(se) - (ll + lb)))
            col += nb
    return total / ntok


_CACHE = {}


def kernel(inp, labels, head_W, head_b, t0_pW, t0_pb, t0_W, t0_b,
           t1_pW, t1_pb, t1_W, t1_b):
    plan, in_maps = make_plan_and_maps(
        inp, labels, head_W, head_b, t0_pW, t0_pb, t0_W, t0_b,
        t1_pW, t1_pb, t1_W, t1_b)
    key = (plan["ntok"], plan["tpc"], plan["cap0"], plan["cap1"],
           plan["use_bias"], T1_BF16, NO_LLDOT, DVE_SLOTS, CONV_ENGINE, WEAVE, CH, PSUM_BUFS)
    if key not in _CACHE:
        _CACHE[key] = build_graph(plan)
    nc = _CACHE[key]
    res = run_bass_kernel_spmd(nc, in_maps, core_ids=list(range(NCORES)))
    outs = [res.results[c]["out"] for c in range(NCORES)]
    loss = assemble_loss(plan, outs)
    return np.asarray(loss, dtype=np.float32)


# revision 24
# speedup vs baseline: 1.9268x; 1.9268x over previous
"""Adaptive-softmax cross-entropy loss on 8 Trainium2 NeuronCores.

Strategy (token-parallel with label-sorted routing):
  * Tokens are sorted by label on the host and dealt round-robin so each
    core takes 512 head tokens plus its share of the cluster-0/1 token
    runs for the two tail softmaxes (low-rank input projections).
  * Each core gathers its token rows from HBM with a transposing
    dma_gather (bf16), landing directly in [K,128]-tile layout for the
    TensorEngine; gpsimd converts to fp8 for DoubleRow matmuls.
  * Streaming softmax over 2048-wide vocab chunks; logits need no
    max-subtraction (|logit| ~ 4).  Per-chunk sum-of-exp is computed on
    TWO engines in parallel, chosen per chunk by a static pattern:
      - ScalarE: Exp activation with fused accum_out.
      - VectorE: Schraudolph fast-exp in bf16-bit space: int16(A*l + B)
        bit-reinterpreted as bf16 ~= exp(l), then a 4x-mode accumulate
        pass.  B tuned so the sum-ratio bias is ~zero.
  * The label logit is NOT extracted from the logit stream.  Instead a
    per-token dot product x . W[:, label] is computed directly: gather
    W^T rows by label (dma_gather) and multiply-accumulate against a
    token-major x gather on VectorE.  For tails the projected hidden
    (256/64-dim, already token-major on chip) is used.
  * Device outputs per-token (sum_exp, label_logit); host finishes with
    log() and the masked mean (tiny O(tokens) work).
"""

import math
import os as _os
from contextlib import ExitStack, nullcontext

import numpy as np
import ml_dtypes

import concourse.bass as bass
import concourse.mybir as mybir
import concourse.tile as tile
from concourse import bacc
from concourse.bass_utils import run_bass_kernel_spmd
from concourse.masks import make_identity

CUTOFFS = (16000, 28000, 36000)
HID = 1024
NCORES = 8
CH = int(_os.environ.get("KERNEL_CH", "1024"))  # vocab chunk width
PSUM_BUFS = int(_os.environ.get("KERNEL_PSUM_BUFS", "4"))
BF16 = mybir.dt.bfloat16
FP8 = mybir.dt.float8e4
F32 = mybir.dt.float32
I16 = mybir.dt.int16
NPBF16 = ml_dtypes.bfloat16
NPFP8 = ml_dtypes.float8_e4m3

# Schraudolph fast-exp in bf16 bit space: bf16_bits(int16(A16*l + B16)) ~ exp(l).
# B16 sits between the round- and trunc-tuned optima so either hardware
# conversion mode keeps the sum-ratio bias under ~1.5e-3.
A16 = float(128.0 / math.log(2.0))
B16 = float(_os.environ.get("KERNEL_B16", "16248.635"))

# which chunk instances the VectorE Schraudolph path takes (i mod 12)
DVE_SLOTS = tuple(
    int(x) for x in _os.environ.get("KERNEL_DVE_SLOTS", "1,3,6,8,10").split(",") if x != ""
)
POOL_SLOTS = tuple(
    int(x) for x in _os.environ.get("KERNEL_POOL_SLOTS", "").split(",") if x != ""
)
T1_BF16 = _os.environ.get("KERNEL_T1_BF16", "0") == "1"  # bisect switch
NO_LLDOT = _os.environ.get("KERNEL_NO_LLDOT", "0") == "1"  # bisect switch
CONV_ENGINE = _os.environ.get("KERNEL_CONV", "pool")  # fp8 convert engine
WEAVE = _os.environ.get("KERNEL_WEAVE", "1") == "1"  # merged vs sequential

# ---------------------------------------------------------------------------
# Workaround for this container's walrus build: CoreV3 codegen accepts only
# ONE embedded sync-wait per instruction, while Tile emits instructions whose
# sync_info carries one wait per producing logical processor. Legalize after
# scheduling: hoist all-but-one wait onto same-engine NoOps inserted directly
# before the instruction (same-engine program order makes this equivalent).
_nop_counter = [0]


def _legalize_sync_waits(nc, max_waits=1):
    for fn in nc.m.functions:
        for blk in fn.blocks:
            insts = blk.instructions
            if not any(
                inst.sync_info is not None
                and inst.sync_info.on_wait
                and len(inst.sync_info.on_wait) > max_waits
                for inst in insts
            ):
                continue
            new = []
            for inst in insts:
                si = inst.sync_info
                waits = list(si.on_wait) if (si is not None and si.on_wait) else []
                if len(waits) > max_waits:
                    for w in waits[:-max_waits]:
                        _nop_counter[0] += 1
                        nop = mybir.InstNoOp(
                            name=f"LW-{_nop_counter[0]}", ins=[], outs=[]
                        )
                        nop.engine = inst.engine
                        nop.sync_info = mybir.SyncInfo(on_wait=[w], on_update=[])
                        nc.register_instruction(nop, overwrite=True)
                        new.append(nop)
                    inst.sync_info = mybir.SyncInfo(
                        on_wait=waits[-max_waits:],
                        on_update=list(si.on_update) if si.on_update else [],
                    )
                new.append(inst)
            blk.instructions = new
# ---------------------------------------------------------------------------


def _cdiv(a, b):
    return (a + b - 1) // b


def _make_chunks(V, prime=False):
    """Vocab chunk boundaries. prime=True starts with narrow chunks so the
    first matmul/exp of the stream fires before the full 2048-wide weight
    chunk has landed (pipeline priming)."""
    chunks = []
    v0 = 0
    if prime and V > 2048:
        chunks = [(0, 1024), (1024, 1024)]
        v0 = 2048
    while v0 < V:
        vw = min(CH, V - v0)
        chunks.append((v0, vw))
        v0 += vw
    return tuple(chunks)


def _wrap_idxs(idxs, num):
    """dma_gather index layout: idx i lives at [i % 16, i // 16], and the
    16-partition block is replicated to all 8 gpsimd cores (128 partitions)."""
    assert num % 16 == 0 and len(idxs) == num
    a = np.asarray(idxs, np.int16).reshape(num // 16, 16).T  # [16, num/16]
    return np.tile(a, (8, 1))  # [128, num/16]


def build_graph(plan, reps=1):
    """One SPMD graph, identical for all 8 cores.

    reps > 1 unrolls the whole kernel body back-to-back inside the NEFF so
    a timing harness can measure marginal (steady-state) per-rep cost,
    cancelling host/dispatch overhead: t = (T(K) - T(1)) / (K - 1)."""
    ntok = plan["ntok"]
    tpc = plan["tpc"]  # head tokens per core (multiple of 128)
    cap0, cap1 = plan["cap0"], plan["cap1"]  # tail token capacity per core
    nbh, nb0, nb1 = tpc // 128, cap0 // 128, cap1 // 128
    chh, ch0l, ch1l = plan["chunks_h"], plan["chunks_0"], plan["chunks_1"]
    nchh, nch0, nch1 = len(chh), len(ch0l), len(ch1l)
    use_bias = plan["use_bias"]
    ncols = plan["ncols"]
    nse = nbh + nb0 + nb1  # sum_exp columns; label-logit columns follow

    nc = bacc.Bacc(num_devices=NCORES)

    xt = nc.declare_dram_parameter("xt", [ntok, HID], BF16, isOutput=False)
    hw = nc.declare_dram_parameter("hw", [HID, 16002], FP8, isOutput=False)
    p0 = nc.declare_dram_parameter("p0", [HID, 256], FP8, isOutput=False)
    w0 = nc.declare_dram_parameter("w0", [256, 12000], FP8, isOutput=False)
    p1 = nc.declare_dram_parameter("p1", [HID, 64], BF16, isOutput=False)
    w1 = nc.declare_dram_parameter("w1", [64, 8000], BF16 if T1_BF16 else FP8, isOutput=False)
    hwT = nc.declare_dram_parameter("hwT", [16002, HID], BF16, isOutput=False)
    w0T = nc.declare_dram_parameter("w0T", [12000, 256], BF16, isOutput=False)
    w1T = nc.declare_dram_parameter("w1T", [8000, 128], BF16, isOutput=False)
    hidx = nc.declare_dram_parameter("hidx", [128, tpc // 16], I16, isOutput=False)
    idx0 = nc.declare_dram_parameter("idx0", [128, cap0 // 16], I16, isOutput=False)
    idx1 = nc.declare_dram_parameter("idx1", [128, cap1 // 16], I16, isOutput=False)
    lidxh = nc.declare_dram_parameter("lidxh", [128, tpc // 16], I16, isOutput=False)
    lidx0 = nc.declare_dram_parameter("lidx0", [128, cap0 // 16], I16, isOutput=False)
    lidx1 = nc.declare_dram_parameter("lidx1", [128, cap1 // 16], I16, isOutput=False)
    if use_bias:
        hb = nc.declare_dram_parameter("hb", [1, 16002], BF16, isOutput=False)
        b0 = nc.declare_dram_parameter("b0", [1, 12000], BF16, isOutput=False)
        b1 = nc.declare_dram_parameter("b1", [1, 8000], BF16, isOutput=False)
        pb0 = nc.declare_dram_parameter("pb0", [1, 256], BF16, isOutput=False)
        pb1 = nc.declare_dram_parameter("pb1", [1, 64], BF16, isOutput=False)
    out = nc.declare_dram_parameter("out", [128, ncols], F32, isOutput=True)
    if _os.environ.get("KERNEL_DEBUG_SE", "0") == "1":
        dbg_hse = nc.declare_dram_parameter("dbg_hse", [128, nchh * 8], F32, isOutput=True)

    Exp = mybir.ActivationFunctionType.Exp
    Mult = mybir.AluOpType.mult
    Add = mybir.AluOpType.add

    with tile.TileContext(nc) as tc:
        with ExitStack() as ctx:
            const = ctx.enter_context(tc.tile_pool(name="const", bufs=1))
            wpool = ctx.enter_context(tc.tile_pool(name="w", bufs=4))
            spool = ctx.enter_context(tc.tile_pool(name="scratch", bufs=1))

            # --- setup: indices / constants (outside rep loop) ---
            idx_sb = {}
            for name, ap, n in (
                ("hidx", hidx, tpc), ("idx0", idx0, cap0), ("idx1", idx1, cap1),
                ("lidxh", lidxh, tpc), ("lidx0", lidx0, cap0), ("lidx1", lidx1, cap1),
            ):
                t = const.tile([128, n // 16], I16, name=f"sb_{name}")
                nc.sync.dma_start(out=t[:, :], in_=ap[:, :])
                idx_sb[name] = t

            identity = const.tile([128, 128], BF16)
            make_identity(nc, identity[:, :])
            ones1 = const.tile([1, 128], BF16)
            nc.vector.memset(ones1[:, :], 1.0)

            def emit_proj_consts():
                nc.sync.dma_start(out=p0_sb[:, :, :],
                                  in_=p0.ap().rearrange("(c p) n -> p c n", p=128))
                nc.sync.dma_start(out=p1_sb[:, :, :],
                                  in_=p1.ap().rearrange("(c p) n -> p c n", p=128))

            bias_sb = {}
            if use_bias:
                for name, ap, n in (
                    ("hb", hb, 16002), ("b0", b0, 12000), ("b1", b1, 8000),
                    ("pb0", pb0, 256), ("pb1", pb1, 64),
                ):
                    t = const.tile([1, n], BF16, tag=f"bias_{name}")
                    nc.sync.dma_start(out=t[:, :], in_=ap[:, :])
                    bias_sb[name] = t

            p0_sb = const.tile([128, 8, 256], FP8)
            p1_sb = const.tile([128, 8, 64], BF16)

            # persistent result tiles (rewritten each rep); xh is block-major
            # [128, nbh, 8K, 128tok] so per-block gathers stay contiguous
            xh = const.tile([128, nbh, 8, 128], BF16)
            x0 = const.tile([128, 8, cap0], BF16)
            x1 = const.tile([128, 8, cap1], BF16)
            xh8 = const.tile([128, nbh, 8, 128], FP8)
            x08 = const.tile([128, 8, cap0], FP8)
            xtm = const.tile([128, nbh, HID], BF16)   # token-major x rows
            wgh = const.tile([128, nbh, HID], BF16)   # W^T rows by head label
            wg0 = const.tile([128, nb0, 256], BF16)
            wg1 = const.tile([128, nb1, 128], BF16)
            hsb0 = const.tile([128, nb0, 256], BF16)  # token-major projected h
            hsb1 = const.tile([128, nb1, 64], BF16)
            hT0 = const.tile([128, 2, cap0], FP8)
            hT1 = const.tile([64, 1, cap1], BF16 if T1_BF16 else FP8)
            hse = const.tile([128, nbh, nchh], F32)
            se0 = const.tile([128, nb0, nch0], F32)
            se1 = const.tile([128, nb1, nch1], F32)
            out_sb = const.tile([128, 2, ncols], F32)

            chunk_counter = [0]
            last_eng = ["D"]
            fillers = []
            conv_eng = nc.gpsimd if CONV_ENGINE == "pool" else nc.vector

            def emit_body(rep):
                par = rep % 2
                # gather this core's token rows (transposed, bf16) and the
                # label-indexed W^T rows. Head gathers go per 128-token block
                # so the first matmuls start early; gpsimd converts to fp8
                # right after each gather.
                with tc.high_priority() if rep == 0 else nullcontext():
                    for b in range(nbh):
                        nc.gpsimd.dma_gather(
                            xh[:, b, :, :], xt[:, :],
                            idx_sb["hidx"][:, b * 8 : (b + 1) * 8],
                            num_idxs=128, num_idxs_reg=128, elem_size=HID,
                            transpose=True,
                        )
                        conv_eng.tensor_copy(xh8[:, b, :, :], xh[:, b, :, :])
                nc.gpsimd.dma_gather(
                    x0[:, :, :], xt[:, :], idx_sb["idx0"][:, :],
                    num_idxs=cap0, num_idxs_reg=cap0, elem_size=HID, transpose=True,
                )
                conv_eng.tensor_copy(x08[:, :, :], x0[:, :, :])
                nc.gpsimd.dma_gather(
                    x1[:, :, :], xt[:, :], idx_sb["idx1"][:, :],
                    num_idxs=cap1, num_idxs_reg=cap1, elem_size=HID, transpose=True,
                )
                nc.gpsimd.dma_gather(
                    xtm[:, :, :], xt[:, :], idx_sb["hidx"][:, :],
                    num_idxs=tpc, num_idxs_reg=tpc, elem_size=HID, transpose=False,
                )
                nc.gpsimd.dma_gather(
                    wgh[:, :, :], hwT[:, :], idx_sb["lidxh"][:, :],
                    num_idxs=tpc, num_idxs_reg=tpc, elem_size=HID, transpose=False,
                )
                nc.gpsimd.dma_gather(
                    wg0[:, :, :], w0T[:, :], idx_sb["lidx0"][:, :],
                    num_idxs=cap0, num_idxs_reg=cap0, elem_size=256, transpose=False,
                )
                nc.gpsimd.dma_gather(
                    wg1[:, :, :], w1T[:, :], idx_sb["lidx1"][:, :],
                    num_idxs=cap1, num_idxs_reg=cap1, elem_size=128, transpose=False,
                )

                # --- tail projections: h = x @ pW (+pb) -> transpose [proj, tok]
                # Emitted AFTER the head stream (see below): engines execute
                # their instruction streams in order, so anything placed
                # before the head matmuls would stall PE on the tail gathers.
                # Projection PSUM tiles borrow the "logits" slots.
                def emit_proj(ppsum):
                    for tb in range(nb0):
                        ph_t = ppsum.tile([128, CH], F32, tag="logits", name="ph_t")
                        ph = ph_t[:, :256]
                        for c2 in range(4):
                            nc.tensor.matmul(
                                ph[:, :],
                                x08[:, 2 * c2 : 2 * c2 + 2, bass.ts(tb, 128)],
                                p0_sb[:, 2 * c2 : 2 * c2 + 2, :],
                                start=(c2 == 0), stop=(c2 == 3 and not use_bias),
                                perf_mode=mybir.MatmulPerfMode.DoubleRow,
                            )
                        if use_bias:
                            nc.tensor.matmul(
                                ph[:, :], ones1[0:1, :], bias_sb["pb0"][0:1, :],
                                start=False, stop=True,
                            )
                        nc.scalar.copy(hsb0[:, tb, :], ph[:, :])
                        for j in range(2):
                            pt_t = ppsum.tile([128, CH], BF16, tag="logits", name="pt_t")
                            pt = pt_t[:, :128]
                            nc.tensor.transpose(pt[:, :], hsb0[:, tb, bass.ts(j, 128)], identity[:, :])
                            nc.scalar.copy(hT0[:, j, bass.ts(tb, 128)], pt[:, :])
                    for tb in range(nb1):
                        ph_t = ppsum.tile([128, CH], F32, tag="logits", name="ph_t")
                        ph = ph_t[:, :64]
                        for c in range(8):
                            nc.tensor.matmul(
                                ph[:, :], x1[:, c, bass.ts(tb, 128)], p1_sb[:, c, :],
                                start=(c == 0), stop=(c == 7 and not use_bias),
                            )
                        if use_bias:
                            nc.tensor.matmul(
                                ph[:, :], ones1[0:1, :], bias_sb["pb1"][0:1, :],
                                start=False, stop=True,
                            )
                        nc.scalar.copy(hsb1[:, tb, :], ph[:, :])
                        pt_t = ppsum.tile([128, CH], BF16, tag="logits", name="pt_t")
                        pt = pt_t[:64, :128]
                        nc.tensor.transpose(pt[:, :], hsb1[:, tb, :], identity[:, :])
                        nc.scalar.copy(hT1[:, 0, bass.ts(tb, 128)], pt[:, :])

                # label-logit dot products: ll[tok] = sum(a[tok,:] * wg[tok,:])
                def lldot_op(a_sb, wg_sb, j, width, col0):
                    def emit():
                        prod = spool.tile([128, HID], BF16, tag="prod", name="prod")
                        nc.vector.tensor_tensor(
                            out=prod[:, :width], in0=a_sb[:, j, :width],
                            in1=wg_sb[:, j, :width], op=Mult)
                        junk2 = spool.tile([128, HID], BF16, tag="junk2", name="junk2")
                        nc.vector.tensor_scalar(
                            out=junk2[:, :width], in0=prod[:, :width],
                            scalar1=1.0, scalar2=None, op0=Mult, op1=Add,
                            accum_out=out_sb[:, par, nse + col0 + j : nse + col0 + j + 1],
                        )
                    return emit

                # --- streaming softmax over vocab chunks, three logical
                # streams (head / tail0 / tail1) interleaved into one merged
                # schedule so PE-heavy head chunks overlap the consumer-heavy
                # tail chunks and the weight DMA never idles.
                def emit_chunk(st, ch, hook=None):
                    KT, kpart, Wr, chunks, nb, se_sb, bias, dr, xsl = (
                        st["KT"], st["kpart"], st["Wr"], st["chunks"],
                        st["nb"], st["se"], st["bias"], st["dr"], st["xsl"])
                    v0, vw = chunks[ch]
                    wdt = FP8 if st["fp8"] else BF16
                    wt = wpool.tile([kpart, KT, vw], wdt, tag="wt", name="wt")
                    if ch == 0 and KT > 1:
                        # split per K-pair so the first matmul needs only
                        # 1/(KT/2) of the chunk's weights (subtile deps)
                        kstep = 2 if dr else 1
                        for c2 in range(KT // kstep):
                            nc.sync.dma_start(
                                out=wt[:, kstep * c2 : kstep * (c2 + 1), :],
                                in_=Wr[:, kstep * c2 : kstep * (c2 + 1), v0 : v0 + vw],
                            )
                    else:
                        nc.sync.dma_start(out=wt[:, :, :], in_=Wr[:, :, v0 : v0 + vw])
                    if hook is not None:
                        hook()
                    for tb in range(nb):
                        ps = lpsum.tile([128, CH], F32, tag="logits", name="ps")
                        if dr:
                            for c2 in range(KT // 2):
                                for s0 in range(0, vw, 512):
                                    sw = min(512, vw - s0)
                                    nc.tensor.matmul(
                                        ps[:, s0 : s0 + sw],
                                        xsl(tb, 2 * c2, 2 * c2 + 2),
                                        wt[:, 2 * c2 : 2 * c2 + 2, s0 : s0 + sw],
                                        start=(c2 == 0),
                                        stop=(c2 == KT // 2 - 1 and bias is None),
                                        perf_mode=mybir.MatmulPerfMode.DoubleRow,
                                    )
                        else:
                            for c in range(KT):
                                for s0 in range(0, vw, 512):
                                    sw = min(512, vw - s0)
                                    nc.tensor.matmul(
                                        ps[:, s0 : s0 + sw],
                                        xsl(tb, c, c + 1),
                                        wt[:, c, s0 : s0 + sw],
                                        start=(c == 0),
                                        stop=(c == KT - 1 and bias is None),
                                    )
                        if bias is not None:
                            for s0 in range(0, vw, 512):
                                sw = min(512, vw - s0)
                                nc.tensor.matmul(
                                    ps[:, s0 : s0 + sw],
                                    ones1[0:1, :],
                                    bias[0:1, v0 + s0 : v0 + s0 + sw],
                                    start=False, stop=True,
                                )
                        # sum-of-exp: ScalarE Exp or VectorE Schraudolph,
                        # chosen per chunk instance by a static pattern.
                        # On an ACT-ACT double (VectorE bubble), slip one
                        # pending DVE filler op (ll-dot / partial reduce)
                        # into the VectorE stream.
                        idx = chunk_counter[0]
                        chunk_counter[0] += 1
                        if (idx % 12) in POOL_SLOTS:
                            i16p = spool.tile([128, CH], I16, tag="i16p", name="i16p")
                            nc.gpsimd.tensor_scalar(
                                out=i16p[:, :vw], in0=ps[:, :vw],
                                scalar1=A16, scalar2=B16, op0=Mult, op1=Add,
                            )
                            junkp = spool.tile([128, CH], BF16, tag="junkp", name="junkp")
                            nc.gpsimd.tensor_scalar(
                                out=junkp[:, :vw], in0=i16p.bitcast(BF16)[:, :vw],
                                scalar1=1.0, scalar2=None, op0=Mult, op1=Add,
                                accum_out=se_sb[:, tb, ch : ch + 1],
                            )
                        elif (idx % 12) in DVE_SLOTS:
                            last_eng[0] = "D"
                            i16t = spool.tile([128, CH], I16, tag="i16", name="i16t")
                            nc.vector.tensor_scalar(
                                out=i16t[:, :vw], in0=ps[:, :vw],
                                scalar1=A16, scalar2=B16, op0=Mult, op1=Add,
                            )
                            junk = spool.tile([128, CH], BF16, tag="junk", name="junk")
                            nc.vector.tensor_scalar(
                                out=junk[:, :vw], in0=i16t.bitcast(BF16)[:, :vw],
                                scalar1=1.0, scalar2=None, op0=Mult, op1=Add,
                                accum_out=se_sb[:, tb, ch : ch + 1],
                            )
                        else:
                            if last_eng[0] == "A" and fillers:
                                fillers.pop(0)()
                            last_eng[0] = "A"
                            ex = spool.tile([128, CH], BF16, tag="ex", name="ex")
                            nc.scalar.activation(
                                ex[:, :vw], ps[:, :vw], Exp,
                                accum_out=se_sb[:, tb, ch : ch + 1],
                            )

                def reduce_op(se_sb, tb, col):
                    def emit():
                        nc.vector.reduce_sum(
                            out=out_sb[:, par, col + tb : col + tb + 1],
                            in_=se_sb[:, tb, :], axis=mybir.AxisListType.X,
                        )
                    return emit

                def blocked_xsl(tb, ca, cb):
                    return xh8[:, tb, ca:cb, :]

                def flat_xsl(xT):
                    return lambda tb, ca, cb: xT[:, ca:cb, bass.ts(tb, 128)]

                sth = dict(KT=8, kpart=128, Wr=hw.ap().rearrange("(c p) n -> p c n", p=128),
                           chunks=chh, nb=nbh, se=hse, bias=bias_sb.get("hb"),
                           dr=True, fp8=True, xsl=blocked_xsl)
                st0 = dict(KT=2, kpart=128, Wr=w0.ap().rearrange("(c p) n -> p c n", p=128),
                           chunks=ch0l, nb=nb0, se=se0, bias=bias_sb.get("b0"),
                           dr=True, fp8=True, xsl=flat_xsl(hT0))
                st1 = dict(KT=1, kpart=64, Wr=w1.ap().rearrange("(c p) n -> p c n", p=64),
                           chunks=ch1l, nb=nb1, se=se1, bias=bias_sb.get("b1"),
                           dr=False, fp8=not T1_BF16, xsl=flat_xsl(hT1))

                with tc.tile_pool(name="lpsum", bufs=PSUM_BUFS, space="PSUM") as lpsum:
                    # priming: two narrow head chunks to get PE going fast
                    emit_chunk(sth, 0, hook=emit_proj_consts if rep == 0 else None)
                    emit_chunk(sth, 1)
                    # projections early: hT0/hT1/hsb ready before their chunks
                    emit_proj(lpsum)
                    # merged schedule: weave tail chunks between head chunks
                    heads = [("h", c) for c in range(2, nchh)]
                    # interleave t1 chunks evenly among t0 chunks
                    tails = []
                    i0 = i1 = 0
                    for i in range(nch0 + nch1):
                        if i1 < ((i + 1) * nch1) // (nch0 + nch1):
                            tails.append(("t1", i1))
                            i1 += 1
                        else:
                            tails.append(("t0", i0))
                            i0 += 1
                    assert i0 == nch0 and i1 == nch1
                    if WEAVE:
                        # weave tails between heads, evenly spread
                        weave = []
                        nt = len(tails)
                        nh = len(heads)
                        ti = 0
                        for i, h in enumerate(heads):
                            weave.append(h)
                            want = (i + 1) * nt // nh
                            while ti < want:
                                weave.append(tails[ti])
                                ti += 1
                        weave.extend(tails[ti:])
                    else:
                        weave = heads + tails

                    if not NO_LLDOT:
                        head_fill = [lldot_op(xtm, wgh, j, HID, 0) for j in range(nbh)]
                        tail_fill = (
                            [lldot_op(hsb0, wg0, j, 256, nbh) for j in range(nb0)]
                            + [lldot_op(hsb1, wg1, j, 64, nbh + nb0) for j in range(nb1)]
                        )
                    else:
                        head_fill, tail_fill = [], []
                    for i, (s, ch) in enumerate(weave):
                        emit_chunk({"h": sth, "t0": st0, "t1": st1}[s], ch)
                        if i == 0 and pending_out:
                            pending_out.pop(0)()
                        if i == 1:
                            # gathers have landed; ll-dots become available
                            fillers.extend(head_fill)
                        elif i == 3:
                            # projections done; tail ll-dots become available
                            fillers.extend(tail_fill)
                        elif s == "t0" and ch == nch0 - 1:
                            fillers.extend(reduce_op(se0, tb, nbh) for tb in range(nb0))
                        elif s == "t1" and ch == nch1 - 1:
                            fillers.extend(reduce_op(se1, tb, nbh + nb0) for tb in range(nb1))
                    for tb in range(nbh):
                        reduce_op(hse, tb, 0)()
                    for f in fillers:
                        f()
                    fillers.clear()

                    pending_out.append(
                        lambda p=par: nc.sync.dma_start(out=out[:, :], in_=out_sb[:, p, :]))
                    if _os.environ.get("KERNEL_DEBUG_SE", "0") == "1":
                        nc.sync.dma_start(
                            out=dbg_hse.ap().rearrange("p (b c) -> p b c", b=nbh),
                            in_=hse[:, :, :])

            pending_out = []
            for _rep in range(reps):
                emit_body(_rep)
            pending_out.pop(0)()

    nc.compile()
    _legalize_sync_waits(nc)
    return nc


def make_plan_and_maps(inp, labels, head_W, head_b, t0_pW, t0_pb, t0_W, t0_b,
                       t1_pW, t1_pb, t1_W, t1_b):
    X = np.ascontiguousarray(np.asarray(inp, np.float32).reshape(-1, HID))
    labels = np.asarray(labels).astype(np.int64).reshape(-1)
    ntok = X.shape[0]
    assert ntok % (NCORES * 128) == 0, ntok

    order = np.argsort(labels, kind="stable")
    slab = labels[order]
    head_labels = labels.copy()
    m0 = (labels >= CUTOFFS[0]) & (labels < CUTOFFS[1])
    m1 = (labels >= CUTOFFS[1]) & (labels < CUTOFFS[2])
    head_labels[m0] = CUTOFFS[0]
    head_labels[m1] = CUTOFFS[0] + 1

    tpc = ntok // NCORES
    # Round-robin deal of sorted tokens keeps per-core work symmetric.
    head_tok = [order[c::NCORES] for c in range(NCORES)]

    def split_cluster(lo, hi):
        toks = order[(slab >= lo) & (slab < hi)]  # sorted by label
        n = len(toks)
        per = _cdiv(max(n, 1), NCORES)
        cap = max(_cdiv(per, 128) * 128, 128)
        chunks, valid = [], []
        for c in range(NCORES):
            chunk = toks[c::NCORES]
            v = len(chunk)
            pad = np.zeros(cap - v, np.int64)
            chunks.append(np.concatenate([chunk, pad]))
            valid.append(v)
        return chunks, valid, cap

    c0_tok, c0_valid, cap0 = split_cluster(CUTOFFS[0], CUTOFFS[1])
    c1_tok, c1_valid, cap1 = split_cluster(CUTOFFS[1], CUTOFFS[2])

    nbh, nb0, nb1 = tpc // 128, cap0 // 128, cap1 // 128
    ncols = 2 * (nbh + nb0 + nb1)

    use_bias = any(
        float(np.abs(np.asarray(b, np.float32)).max()) > 0
        for b in (head_b, t0_b, t1_b, t0_pb, t1_pb)
    )

    chunks_h = _make_chunks(16002, prime=True)
    chunks_0 = _make_chunks(12000)
    chunks_1 = _make_chunks(8000)

    plan = dict(ntok=ntok, tpc=tpc, cap0=cap0, cap1=cap1, ncols=ncols,
                use_bias=use_bias, head_tok=head_tok, c0_tok=c0_tok,
                c1_tok=c1_tok, c0_valid=c0_valid, c1_valid=c1_valid,
                labels=labels, head_labels=head_labels,
                head_b=np.asarray(head_b, np.float64),
                t0_b=np.asarray(t0_b, np.float64),
                t1_b=np.asarray(t1_b, np.float64),
                chunks_h=chunks_h, chunks_0=chunks_0, chunks_1=chunks_1)

    Xb = X.astype(NPBF16)
    W0 = np.asarray(t0_W, np.float32)
    W1 = np.asarray(t1_W, np.float32)
    HW = np.asarray(head_W, np.float32)
    w1T = np.zeros((8000, 128), np.float32)
    w1T[:, :64] = W1.T
    shared = {
        "xt": Xb,
        "hw": HW.astype(NPFP8),
        "p0": np.asarray(t0_pW, np.float32).astype(NPFP8),
        "w0": W0.astype(NPFP8),
        "p1": np.asarray(t1_pW, np.float32).astype(NPBF16),
        "w1": W1.astype(NPBF16 if T1_BF16 else NPFP8),
        "hwT": np.ascontiguousarray(HW.T).astype(NPBF16),
        "w0T": np.ascontiguousarray(W0.T).astype(NPBF16),
        "w1T": w1T.astype(NPBF16),
    }
    if use_bias:
        shared["hb"] = np.asarray(head_b, np.float32).astype(NPBF16)[None, :]
        shared["b0"] = np.asarray(t0_b, np.float32).astype(NPBF16)[None, :]
        shared["b1"] = np.asarray(t1_b, np.float32).astype(NPBF16)[None, :]
        shared["pb0"] = np.asarray(t0_pb, np.float32).astype(NPBF16)[None, :]
        shared["pb1"] = np.asarray(t1_pb, np.float32).astype(NPBF16)[None, :]

    in_maps = []
    for c in range(NCORES):
        m = dict(shared)
        m["hidx"] = _wrap_idxs(head_tok[c], tpc)
        m["idx0"] = _wrap_idxs(c0_tok[c], cap0)
        m["idx1"] = _wrap_idxs(c1_tok[c], cap1)
        m["lidxh"] = _wrap_idxs(head_labels[head_tok[c]], tpc)

        def _lab_idx(tok_list, valid, lo, cap):
            v = np.zeros(cap, np.int64)
            v[:valid] = labels[tok_list[:valid]] - lo
            return _wrap_idxs(v, cap)

        m["lidx0"] = _lab_idx(c0_tok[c], c0_valid[c], CUTOFFS[0], cap0)
        m["lidx1"] = _lab_idx(c1_tok[c], c1_valid[c], CUTOFFS[1], cap1)
        in_maps.append(m)
    return plan, in_maps


def assemble_loss(plan, outs):
    """outs: list of per-core [128, ncols] f32 arrays -> mean loss (f64)."""
    ntok = plan["ntok"]
    labels = plan["labels"]
    tpc = plan["tpc"]
    nbh = tpc // 128
    nb0 = plan["cap0"] // 128
    nb1 = plan["cap1"] // 128
    nse = nbh + nb0 + nb1
    total = 0.0
    for c in range(NCORES):
        o = np.asarray(outs[c], np.float64)
        col = 0
        for tok_list, nb, valid, bias, blab in (
            (plan["head_tok"][c], nbh, tpc, plan["head_b"], plan["head_labels"]),
            (plan["c0_tok"][c], nb0, plan["c0_valid"][c], plan["t0_b"],
             labels - CUTOFFS[0]),
            (plan["c1_tok"][c], nb1, plan["c1_valid"][c], plan["t1_b"],
             labels - CUTOFFS[1]),
        ):
            # block b partition p <-> token b*128+p of tok_list
            se = o[:, col : col + nb].T.reshape(-1)[:valid]
            ll = o[:, nse + col : nse + col + nb].T.reshape(-1)[:valid]
            toks = tok_list[:valid]
            w = (labels[toks] != 0).astype(np.float64)
            lb = bias[blab[toks]] if bias.ndim else 0.0
            total += float(np.dot(w, np.log(se) - (ll + lb)))
            col += nb
    return total / ntok


_CACHE = {}


def kernel(inp, labels, head_W, head_b, t0_pW, t0_pb, t0_W, t0_b,
           t1_pW, t1_pb, t1_W, t1_b):
    plan, in_maps = make_plan_and_maps(
        inp, labels, head_W, head_b, t0_pW, t0_pb, t0_W, t0_b,
        t1_pW, t1_pb, t1_W, t1_b)
    key = (plan["ntok"], plan["tpc"], plan["cap0"], plan["cap1"],
           plan["use_bias"], T1_BF16, NO_LLDOT, DVE_SLOTS, POOL_SLOTS, CONV_ENGINE, WEAVE, CH, PSUM_BUFS)
    if key not in _CACHE:
        _CACHE[key] = build_graph(plan)
    nc = _CACHE[key]
    res = run_bass_kernel_spmd(nc, in_maps, core_ids=list(range(NCORES)))
    outs = [res.results[c]["out"] for c in range(NCORES)]
    loss = assemble_loss(plan, outs)
    return np.asarray(loss, dtype=np.float32)
